# revision 10
# baseline (speedup 1.0000x reference)
"""Trainium2 Bass kernel: batched complex-waveform similarity.

Math: reference computes
    bank = ifft_ortho(freq)                # [T, L] complex
    score = rx @ conj(bank).T              # [B, T] complex
    sim   = (score.re^2 + score.im^2) / temperature

Since the ortho DFT is unitary,  score = fft_ortho(rx) @ conj(freq).T.
So the kernel never builds the bank: it DFTs rx via a 128x128 bf16
matmul, then runs the big complex GEMM [B,L]x[L,T] in bf16 with fp32
PSUM accumulation and a fused squared-magnitude epilogue.

With f1=fr, f2=-(fr+fi), f3=fi-fr and xs=xr+xi, the complex GEMM is
    im = xs*f1 + xr*f2;  re = xs*f1 + xi*f3
(4 matmuls per [128,1024] supertile half-pair; consecutive matmuls
share the stationary operand, which lets PE issue at ~215ns/matmul
instead of ~259). im is accumulated first so ACT's Square(im) overlaps
the re matmuls; DVE then fuses out = (re^2 + t2) * (1/temp).

PSUM: four [128,1024] tiles (reA/imA for even supertiles, reB/imB for
odd) = all 8 banks; a supertile's banks are reused 2 supertiles later,
which gives the epilogue a full cadence to drain.

Sharding: data-parallel over the rx batch dim across 8 NeuronCores;
freq planes are replicated on every core.
"""

import numpy as np
import ml_dtypes

B = 8192
T = 8192
L = 128
NCORES = 8
BPC = B // NCORES  # batch rows per core

_BF16 = ml_dtypes.bfloat16

_CACHE = {}


# --------------------------------------------------------------------------- #
# Custom DVE op: out = (Src0^2 + Src1) * C0
# (Src0 = re from PSUM, Src1 = im^2 staged by ScalarE, C0 = 1/temperature)
# --------------------------------------------------------------------------- #
def _get_sqadd_op():
    import concourse.dve_ops as dve_ops
    from concourse.dve_spec import Spec, Src0, Src1, C0, sq, lower, _has_src1
    from concourse.dve_uop import DveOpSpec

    name = "SQ_ADD_SCALE_ANT"
    for op in dve_ops.OPS:
        if op.name == name:
            return op

    spec = Spec(
        body=(sq(Src0) + Src1) * C0,
        reference=lambda in0, in1, s0, s1, imm2: (
            (in0.astype(np.float32) ** 2 + in1.astype(np.float32)) * s0
        ).astype(np.float32),
    )
    opcode = dve_ops._CUSTOM_DVE_ROW_BASE + len(dve_ops.OPS)
    assert opcode < 0x20
    shas = {}
    for ver in ("v3", "v4"):
        compiled = DveOpSpec(
            name=name, opcode=opcode, uops=lower(spec, ver=ver), rd1_en=_has_src1(spec)
        )
        shas[ver] = compiled.sha(ver)
    op = dve_ops.DveOp(name, spec, subdim=False, uops_sha=shas)
    dve_ops.OPS.append(op)
    dve_ops.CUSTOM_DVE_SPECS[name] = spec
    dve_ops._SUB_OPCODE_FOR_NAME[name] = opcode
    return op


# --------------------------------------------------------------------------- #
# Bass program (one SPMD NeuronCore)
# --------------------------------------------------------------------------- #
def build_nc(bpc=BPC, t=T, debug=False):
    from contextlib import ExitStack

    import concourse.bacc as bacc
    import concourse.bass as bass
    import concourse.mybir as mybir
    import concourse.tile as tile

    f32 = mybir.dt.float32
    bf16 = mybir.dt.bfloat16
    sqadd = _get_sqadd_op()

    ST = 1024  # supertile width: 2 PSUM banks
    OBW = 2048  # out staging tile: 2 supertiles per 1 MiB DMA
    n_st = t // ST
    n_m = bpc // 128
    assert bpc % 512 == 0 and t % ST == 0

    nc = bacc.Bacc("TRN2", target_bir_lowering=False, debug=debug, num_devices=NCORES)

    rxt_r = nc.dram_tensor("rxt_r", [L, bpc], bf16, kind="ExternalInput")
    rxt_i = nc.dram_tensor("rxt_i", [L, bpc], bf16, kind="ExternalInput")
    fq1 = nc.dram_tensor("fq1", [L, t], bf16, kind="ExternalInput")  # fr
    fq2 = nc.dram_tensor("fq2", [L, t], bf16, kind="ExternalInput")  # -(fr+fi)
    fq3 = nc.dram_tensor("fq3", [L, t], bf16, kind="ExternalInput")  # fi-fr
    w_r = nc.dram_tensor("w_r", [L, L], bf16, kind="ExternalInput")
    w_i = nc.dram_tensor("w_i", [L, L], bf16, kind="ExternalInput")
    w_ni = nc.dram_tensor("w_ni", [L, L], bf16, kind="ExternalInput")
    temp = nc.dram_tensor("temp", [128, 1], f32, kind="ExternalInput")
    out = nc.dram_tensor("out", [bpc, t], f32, kind="ExternalOutput")

    with tile.TileContext(nc) as tc, ExitStack() as ctx:
        consts = ctx.enter_context(tc.tile_pool(name="consts", bufs=1))
        psum = ctx.enter_context(
            tc.tile_pool(name="psum", bufs=1, space=bass.MemorySpace.PSUM)
        )
        sq_pool = ctx.enter_context(tc.tile_pool(name="sq", bufs=4))
        out_pool = ctx.enter_context(tc.tile_pool(name="ob", bufs=4))

        def ptile(k, which):  # which in ("re", "im")
            tag = f"{which}{'AB'[k % 2]}"
            return psum.tile([128, ST], f32, tag=tag, name=f"ps_{tag}")

        # ---- PE warmup ------------------------------------------------ #
        # Dependency-free matmuls run during the input-DMA window so the
        # HAM clock gate is at 8/8 when the real matmuls start (~12.5us).
        warm_w = consts.tile([128, 128], bf16)
        nc.gpsimd.memset(warm_w[:], 0)
        # dummy Pool tensor-op: loads the Q7 library now, not on the
        # critical xs = xr + xi adds after the DFT
        nc.gpsimd.tensor_add(warm_w[:, 0:1], warm_w[:, 0:1], warm_w[:, 0:1])
        warm_ps = ptile(0, "re")
        for _ in range(44):
            nc.tensor.matmul(warm_ps[:, 0:128], warm_w[:], warm_w[:], start=True, stop=True)

        # ---- load constants / inputs ---------------------------------- #
        # rx first on the SP ring; W + temp first on the ScalarE ring (in
        # parallel), then the freq planes interleave across both rings so
        # group g lands well before the m=0 walk reaches it.
        rxr_sb = consts.tile([L, bpc], bf16)
        nc.sync.dma_start(rxr_sb[:], rxt_r[:, :])
        rxi_sb = consts.tile([L, bpc], bf16)
        nc.sync.dma_start(rxi_sb[:], rxt_i[:, :])
        wr_sb = consts.tile([L, L], bf16)
        nc.sync.dma_start(wr_sb[:], w_r[:, :])
        wni_sb = consts.tile([L, L], bf16)
        nc.sync.dma_start(wni_sb[:], w_ni[:, :])
        wi_sb = consts.tile([L, L], bf16)
        nc.sync.dma_start(wi_sb[:], w_i[:, :])
        temp_sb = consts.tile([128, 1], f32)
        nc.scalar.dma_start(temp_sb[:], temp[:, :])

        f1_sb, f2_sb, f3_sb = [], [], []
        for g in range(n_st):
            gs = slice(g * ST, (g + 1) * ST)
            for plane, dram, lst in (
                ("f1", fq1, f1_sb),
                ("f2", fq2, f2_sb),
                ("f3", fq3, f3_sb),
            ):
                ft = consts.tile([L, ST], bf16, tag=f"{plane}g{g}", name=f"{plane}g{g}")
                nc.sync.dma_start(ft[:], dram[:, gs])
                lst.append(ft)

        invt_sb = consts.tile([128, 1], f32)
        nc.vector.reciprocal(invt_sb[:], temp_sb[:])

        # ---- DFT of rx (bf16): rxfT = W @ rxT ------------------------- #
        # W symmetric, so PE's lhsT is W itself.
        # rxfT_r = Wr@rxT_r - Wi@rxT_i ; rxfT_i = Wr@rxT_i + Wi@rxT_r
        # Evictions: ACT -> xr, DVE -> xi, Pool -> xs = xr + xi.
        xr_sb = consts.tile([L, bpc], bf16)
        xi_sb = consts.tile([L, bpc], bf16)
        xs_sb = consts.tile([L, bpc], bf16)
        for ci, c0 in enumerate(range(0, bpc, 512)):
            cs = slice(c0, c0 + 512)
            pr = ptile(1, "re")
            nc.tensor.matmul(pr[:, 0:512], wr_sb[:], rxr_sb[:, cs], start=True, stop=False)
            nc.tensor.matmul(pr[:, 0:512], wni_sb[:], rxi_sb[:, cs], start=False, stop=True)
            pi = ptile(1, "im")
            nc.tensor.matmul(pi[:, 0:512], wr_sb[:], rxi_sb[:, cs], start=True, stop=False)
            nc.tensor.matmul(pi[:, 0:512], wi_sb[:], rxr_sb[:, cs], start=False, stop=True)
            # chunk 0's evictions split 256-wide so xs[:, :128] (all the
            # m=0 row needs) is ready as early as possible
            splits = (0, 256, 512) if ci == 0 else (0, 512)
            for e0, e1 in zip(splits, splits[1:]):
                es = slice(c0 + e0, c0 + e1)
                ps = slice(e0, e1)
                nc.scalar.copy(xr_sb[:, es], pr[:, ps])
                if ci == 0:
                    nc.vector.tensor_copy(xi_sb[:, es], pi[:, ps])
                else:
                    nc.scalar.copy(xi_sb[:, es], pi[:, ps])
                nc.gpsimd.tensor_add(xs_sb[:, es], xr_sb[:, es], xi_sb[:, es])

        # ---- main complex GEMM + fused |.|^2 epilogue ----------------- #
        # Pure 4-matmul complex GEMM per [128,1024] supertile, im banks
        # first so ACT's square overlaps the re matmuls, then DVE's fused
        # (re^2 + t2) * (1/temp) drains both banks.  Each supertile's
        # banks recycle 2 supertiles later (reA/imA vs reB/imB), which
        # just covers the mm->square->sqadd chain.
        state = {"ob": None}

        def supertile(idx, m, s):
            ms = slice(m * 128, (m + 1) * 128)
            pim = ptile(idx, "im")
            pre = ptile(idx, "re")
            for h in (0, 1):
                hs = slice(h * 512, (h + 1) * 512)
                nc.tensor.matmul(
                    pim[:, hs], xs_sb[:, ms], f1_sb[s][:, hs],
                    start=True, stop=True, skip_group_check=True,
                )
            for h in (0, 1):
                hs = slice(h * 512, (h + 1) * 512)
                nc.tensor.matmul(
                    pim[:, hs], xr_sb[:, ms], f2_sb[s][:, hs],
                    start=False, stop=True, skip_group_check=True,
                )
            t2 = sq_pool.tile([128, ST], f32, name="t2")
            last_st = m == n_m - 1 and s == n_st - 1
            if last_st:
                nc.scalar.square(t2[:, 0:512], pim[:, 0:512])
                nc.scalar.square(t2[:, 512:1024], pim[:, 512:1024])
            else:
                nc.scalar.square(t2[:], pim[:])
            for h in (0, 1):
                hs = slice(h * 512, (h + 1) * 512)
                nc.tensor.matmul(
                    pre[:, hs], xs_sb[:, ms], f1_sb[s][:, hs],
                    start=True, stop=True, skip_group_check=True,
                )
            for h in (0, 1):
                hs = slice(h * 512, (h + 1) * 512)
                nc.tensor.matmul(
                    pre[:, hs], xi_sb[:, ms], f3_sb[s][:, hs],
                    start=False, stop=True, skip_group_check=True,
                )
            o = s % (OBW // ST)
            if o == 0:
                state["ob"] = out_pool.tile([128, OBW], f32, name="ob")
            ob = state["ob"]
            if last_st:
                for h in (0, 1):
                    hs = slice(o * ST + h * 512, o * ST + (h + 1) * 512)
                    nc.vector._custom_dve(
                        sqadd,
                        out=ob[:, hs],
                        in0=pre[:, h * 512 : (h + 1) * 512],
                        in1=t2[:, h * 512 : (h + 1) * 512],
                        s0=invt_sb[:],
                    )
                    nc.scalar.dma_start(
                        out[ms, s * ST + h * 512 : s * ST + (h + 1) * 512], ob[:, hs]
                    )
                return
            nc.vector._custom_dve(
                sqadd,
                out=ob[:, o * ST : (o + 1) * ST],
                in0=pre[:],
                in1=t2[:],
                s0=invt_sb[:],
            )
            if m == n_m - 1 and s >= n_st - OBW // ST:
                # final pair: per-supertile DMAs so the last transfer is
                # smaller and the kernel-exit barrier waits less
                nc.scalar.dma_start(
                    out[ms, s * ST : (s + 1) * ST], ob[:, o * ST : (o + 1) * ST]
                )
            elif o == OBW // ST - 1:
                s0 = s - o
                # m=0 and the last m-row ride the ScalarE ring: early on
                # the SP ring must finish the freq loads before the m=0
                # walk catches up, and at the end the exit barrier would
                # wait on the SP ring's ~3-transfer backlog
                oeng = nc.scalar if m == n_m - 1 or m == 0 else nc.sync
                oeng.dma_start(out[ms, s0 * ST : s0 * ST + OBW], ob[:])

        idx = 0
        for m in range(n_m):
            for s in range(n_st):
                supertile(idx, m, s)
                idx += 1

    nc.compile()
    return nc


def _host_prep(rx_real, rx_imag, freq_real, freq_imag, temperature, bpc=BPC, t=T):
    """Layout marshaling only: shard/transpose/cast inputs for the cores."""
    lk = np.outer(np.arange(L), np.arange(L)).astype(np.float64)
    w = np.exp(-2j * np.pi * lk / L) / np.sqrt(L)  # ortho DFT matrix (symmetric)
    w_r = np.ascontiguousarray(w.real.astype(np.float32).astype(_BF16))
    w_i = np.ascontiguousarray(w.imag.astype(np.float32).astype(_BF16))
    w_ni = np.ascontiguousarray((-w.imag).astype(np.float32).astype(_BF16))

    fr = freq_real[:t].T.astype(np.float32)  # [L, T]
    fi = freq_imag[:t].T.astype(np.float32)
    fq1 = np.ascontiguousarray(fr.astype(_BF16))
    fq2 = np.ascontiguousarray((-(fr + fi)).astype(_BF16))
    fq3 = np.ascontiguousarray((fi - fr).astype(_BF16))
    temp_col = np.full((128, 1), np.asarray(temperature), np.float32)

    rxt_r = np.asarray(rx_real, np.float32).T.astype(_BF16)  # [L, B]
    rxt_i = np.asarray(rx_imag, np.float32).T.astype(_BF16)

    in_maps = []
    for c in range(NCORES):
        cs = slice(c * bpc, (c + 1) * bpc)
        in_maps.append(
            {
                "rxt_r": np.ascontiguousarray(rxt_r[:, cs]),
                "rxt_i": np.ascontiguousarray(rxt_i[:, cs]),
                "fq1": fq1,
                "fq2": fq2,
                "fq3": fq3,
                "w_r": w_r,
                "w_i": w_i,
                "w_ni": w_ni,
                "temp": temp_col,
            }
        )
    return in_maps


def kernel(rx_real, rx_imag, freq_real, freq_imag, temperature):
    from concourse.bass_utils import run_bass_kernel_spmd

    if "nc" not in _CACHE:
        _CACHE["nc"] = build_nc()
    nc = _CACHE["nc"]

    in_maps = _host_prep(rx_real, rx_imag, freq_real, freq_imag, temperature)
    res = run_bass_kernel_spmd(nc, in_maps, core_ids=list(range(NCORES)))
    _CACHE["last_result"] = res
    return np.concatenate([r["out"] for r in res.results], axis=0)


# revision 11
# speedup vs baseline: 1.0282x; 1.0282x over previous
"""Trainium2 Bass kernel: batched complex-waveform similarity.

Math: reference computes
    bank = ifft_ortho(freq)                # [T, L] complex
    score = rx @ conj(bank).T              # [B, T] complex
    sim   = (score.re^2 + score.im^2) / temperature

Since the ortho DFT is unitary,  score = fft_ortho(rx) @ conj(freq).T.
So the kernel never builds the bank: it DFTs rx via a 128x128 bf16
matmul, then runs the big complex GEMM [B,L]x[L,T] in bf16 with fp32
PSUM accumulation and a fused squared-magnitude epilogue.

With f1=fr, f2=-(fr+fi), f3=fi-fr and xs=xr+xi, the complex GEMM is
    im = xs*f1 + xr*f2;  re = xs*f1 + xi*f3
(4 matmuls per [128,1024] supertile half-pair; consecutive matmuls
share the stationary operand, which lets PE issue at ~215ns/matmul
instead of ~259). im is accumulated first so ACT's Square(im) overlaps
the re matmuls; DVE then fuses out = (re^2 + t2) * (1/temp).

PSUM: four [128,1024] tiles (reA/imA for even supertiles, reB/imB for
odd) = all 8 banks; a supertile's banks are reused 2 supertiles later,
which gives the epilogue a full cadence to drain.

Sharding: data-parallel over the rx batch dim across 8 NeuronCores;
freq planes are replicated on every core.
"""

import numpy as np
import ml_dtypes

B = 8192
T = 8192
L = 128
NCORES = 8
BPC = B // NCORES  # batch rows per core

_BF16 = ml_dtypes.bfloat16

_CACHE = {}


# --------------------------------------------------------------------------- #
# Custom DVE op: out = (Src0^2 + Src1) * C0
# (Src0 = re from PSUM, Src1 = im^2 staged by ScalarE, C0 = 1/temperature)
# --------------------------------------------------------------------------- #
def _get_sqadd_op():
    import concourse.dve_ops as dve_ops
    from concourse.dve_spec import Spec, Src0, Src1, C0, sq, lower, _has_src1
    from concourse.dve_uop import DveOpSpec

    name = "SQ_ADD_SCALE_ANT"
    for op in dve_ops.OPS:
        if op.name == name:
            return op

    spec = Spec(
        body=(sq(Src0) + Src1) * C0,
        reference=lambda in0, in1, s0, s1, imm2: (
            (in0.astype(np.float32) ** 2 + in1.astype(np.float32)) * s0
        ).astype(np.float32),
    )
    opcode = dve_ops._CUSTOM_DVE_ROW_BASE + len(dve_ops.OPS)
    assert opcode < 0x20
    shas = {}
    for ver in ("v3", "v4"):
        compiled = DveOpSpec(
            name=name, opcode=opcode, uops=lower(spec, ver=ver), rd1_en=_has_src1(spec)
        )
        shas[ver] = compiled.sha(ver)
    op = dve_ops.DveOp(name, spec, subdim=False, uops_sha=shas)
    dve_ops.OPS.append(op)
    dve_ops.CUSTOM_DVE_SPECS[name] = spec
    dve_ops._SUB_OPCODE_FOR_NAME[name] = opcode
    return op


# --------------------------------------------------------------------------- #
# Bass program (one SPMD NeuronCore)
# --------------------------------------------------------------------------- #
def build_nc(bpc=BPC, t=T, debug=False):
    from contextlib import ExitStack

    import concourse.bacc as bacc
    import concourse.bass as bass
    import concourse.mybir as mybir
    import concourse.tile as tile

    f32 = mybir.dt.float32
    bf16 = mybir.dt.bfloat16
    sqadd = _get_sqadd_op()

    ST = 1024  # supertile width: 2 PSUM banks
    OBW = 2048  # out staging tile: 2 supertiles per 1 MiB DMA
    n_st = t // ST
    n_m = bpc // 128
    assert bpc % 512 == 0 and t % ST == 0

    nc = bacc.Bacc("TRN2", target_bir_lowering=False, debug=debug, num_devices=NCORES)

    rxt_r = nc.dram_tensor("rxt_r", [L, bpc], bf16, kind="ExternalInput")
    rxt_i = nc.dram_tensor("rxt_i", [L, bpc], bf16, kind="ExternalInput")
    fq1 = nc.dram_tensor("fq1", [L, t], bf16, kind="ExternalInput")  # fr
    fq3 = nc.dram_tensor("fq3", [L, t], bf16, kind="ExternalInput")  # fi-fr
    w_r = nc.dram_tensor("w_r", [L, L], bf16, kind="ExternalInput")
    w_i = nc.dram_tensor("w_i", [L, L], bf16, kind="ExternalInput")
    w_ni = nc.dram_tensor("w_ni", [L, L], bf16, kind="ExternalInput")
    temp = nc.dram_tensor("temp", [128, 1], f32, kind="ExternalInput")
    out = nc.dram_tensor("out", [bpc, t], f32, kind="ExternalOutput")

    with tile.TileContext(nc) as tc, ExitStack() as ctx:
        consts = ctx.enter_context(tc.tile_pool(name="consts", bufs=1))
        psum = ctx.enter_context(
            tc.tile_pool(name="psum", bufs=1, space=bass.MemorySpace.PSUM)
        )
        sq_pool = ctx.enter_context(tc.tile_pool(name="sq", bufs=4))
        out_pool = ctx.enter_context(tc.tile_pool(name="ob", bufs=4))

        def ptile(k, which):  # which in ("re", "im")
            tag = f"{which}{'AB'[k % 2]}"
            return psum.tile([128, ST], f32, tag=tag, name=f"ps_{tag}")

        # ---- PE warmup ------------------------------------------------ #
        # Dependency-free matmuls run during the input-DMA window so the
        # HAM clock gate is at 8/8 when the real matmuls start (~12.5us).
        warm_w = consts.tile([128, 128], bf16)
        nc.gpsimd.memset(warm_w[:], 0)
        # dummy Pool tensor-op: loads the Q7 library now, not on the
        # critical xs = xr + xi adds after the DFT
        nc.gpsimd.tensor_add(warm_w[:, 0:1], warm_w[:, 0:1], warm_w[:, 0:1])
        warm_ps = ptile(0, "re")
        for _ in range(44):
            nc.tensor.matmul(warm_ps[:, 0:128], warm_w[:], warm_w[:], start=True, stop=True)

        # ---- load constants / inputs ---------------------------------- #
        # rx first on the SP ring; W + temp first on the ScalarE ring (in
        # parallel), then the freq planes interleave across both rings so
        # group g lands well before the m=0 walk reaches it.
        rxr_sb = consts.tile([L, bpc], bf16)
        nc.sync.dma_start(rxr_sb[:], rxt_r[:, :])
        rxi_sb = consts.tile([L, bpc], bf16)
        nc.sync.dma_start(rxi_sb[:], rxt_i[:, :])
        wr_sb = consts.tile([L, L], bf16)
        nc.sync.dma_start(wr_sb[:], w_r[:, :])
        wni_sb = consts.tile([L, L], bf16)
        nc.sync.dma_start(wni_sb[:], w_ni[:, :])
        wi_sb = consts.tile([L, L], bf16)
        nc.sync.dma_start(wi_sb[:], w_i[:, :])
        temp_sb = consts.tile([128, 1], f32)
        nc.scalar.dma_start(temp_sb[:], temp[:, :])

        f1_sb, f2_sb, f3_sb = [], [], []
        for g in range(n_st):
            gs = slice(g * ST, (g + 1) * ST)
            for plane, dram, lst in (("f1", fq1, f1_sb), ("f3", fq3, f3_sb)):
                ft = consts.tile([L, ST], bf16, tag=f"{plane}g{g}", name=f"{plane}g{g}")
                nc.sync.dma_start(ft[:], dram[:, gs])
                lst.append(ft)
            # f2 = -(fr+fi) = -(2*f1 + f3), computed on DVE once f1/f3 land
            f2_sb.append(consts.tile([L, ST], bf16, tag=f"f2g{g}", name=f"f2g{g}"))

        def emit_f2(g):
            nc.vector.scalar_tensor_tensor(
                f2_sb[g][:], f1_sb[g][:], -2.0, f3_sb[g][:],
                mybir.AluOpType.mult, mybir.AluOpType.subtract,
            )

        invt_sb = consts.tile([128, 1], f32)
        nc.vector.reciprocal(invt_sb[:], temp_sb[:])

        # ---- DFT of rx (bf16): rxfT = W @ rxT ------------------------- #
        # W symmetric, so PE's lhsT is W itself.
        # rxfT_r = Wr@rxT_r - Wi@rxT_i ; rxfT_i = Wr@rxT_i + Wi@rxT_r
        # Evictions: ACT -> xr, DVE -> xi, Pool -> xs = xr + xi.
        xr_sb = consts.tile([L, bpc], bf16)
        xi_sb = consts.tile([L, bpc], bf16)
        xs_sb = consts.tile([L, bpc], bf16)
        for ci, c0 in enumerate(range(0, bpc, 512)):
            cs = slice(c0, c0 + 512)
            pr = ptile(1, "re")
            nc.tensor.matmul(pr[:, 0:512], wr_sb[:], rxr_sb[:, cs], start=True, stop=False)
            nc.tensor.matmul(pr[:, 0:512], wni_sb[:], rxi_sb[:, cs], start=False, stop=True)
            pi = ptile(1, "im")
            nc.tensor.matmul(pi[:, 0:512], wr_sb[:], rxi_sb[:, cs], start=True, stop=False)
            nc.tensor.matmul(pi[:, 0:512], wi_sb[:], rxr_sb[:, cs], start=False, stop=True)
            # chunk 0's evictions split 256-wide so xs[:, :128] (all the
            # m=0 row needs) is ready as early as possible
            splits = (0, 256, 512) if ci == 0 else (0, 512)
            for e0, e1 in zip(splits, splits[1:]):
                es = slice(c0 + e0, c0 + e1)
                ps = slice(e0, e1)
                nc.scalar.copy(xr_sb[:, es], pr[:, ps])
                if ci == 0:
                    nc.vector.tensor_copy(xi_sb[:, es], pi[:, ps])
                else:
                    nc.scalar.copy(xi_sb[:, es], pi[:, ps])
                nc.gpsimd.tensor_add(xs_sb[:, es], xr_sb[:, es], xi_sb[:, es])
        emit_f2(0)
        emit_f2(1)

        # ---- main complex GEMM + fused |.|^2 epilogue ----------------- #
        # Pure 4-matmul complex GEMM per [128,1024] supertile, im banks
        # first so ACT's square overlaps the re matmuls, then DVE's fused
        # (re^2 + t2) * (1/temp) drains both banks.  Each supertile's
        # banks recycle 2 supertiles later (reA/imA vs reB/imB), which
        # just covers the mm->square->sqadd chain.
        state = {"ob": None}

        def supertile(idx, m, s):
            ms = slice(m * 128, (m + 1) * 128)
            pim = ptile(idx, "im")
            pre = ptile(idx, "re")
            for h in (0, 1):
                hs = slice(h * 512, (h + 1) * 512)
                nc.tensor.matmul(
                    pim[:, hs], xs_sb[:, ms], f1_sb[s][:, hs],
                    start=True, stop=True, skip_group_check=True,
                )
            for h in (0, 1):
                hs = slice(h * 512, (h + 1) * 512)
                nc.tensor.matmul(
                    pim[:, hs], xr_sb[:, ms], f2_sb[s][:, hs],
                    start=False, stop=True, skip_group_check=True,
                )
            t2 = sq_pool.tile([128, ST], f32, name="t2")
            last_st = m == n_m - 1 and s == n_st - 1
            if last_st:
                nc.scalar.square(t2[:, 0:512], pim[:, 0:512])
                nc.scalar.square(t2[:, 512:1024], pim[:, 512:1024])
            else:
                nc.scalar.square(t2[:], pim[:])
            for h in (0, 1):
                hs = slice(h * 512, (h + 1) * 512)
                nc.tensor.matmul(
                    pre[:, hs], xs_sb[:, ms], f1_sb[s][:, hs],
                    start=True, stop=True, skip_group_check=True,
                )
            for h in (0, 1):
                hs = slice(h * 512, (h + 1) * 512)
                nc.tensor.matmul(
                    pre[:, hs], xi_sb[:, ms], f3_sb[s][:, hs],
                    start=False, stop=True, skip_group_check=True,
                )
            o = s % (OBW // ST)
            if o == 0:
                state["ob"] = out_pool.tile([128, OBW], f32, name="ob")
            ob = state["ob"]
            if last_st:
                for h in (0, 1):
                    hs = slice(o * ST + h * 512, o * ST + (h + 1) * 512)
                    nc.vector._custom_dve(
                        sqadd,
                        out=ob[:, hs],
                        in0=pre[:, h * 512 : (h + 1) * 512],
                        in1=t2[:, h * 512 : (h + 1) * 512],
                        s0=invt_sb[:],
                    )
                    nc.scalar.dma_start(
                        out[ms, s * ST + h * 512 : s * ST + (h + 1) * 512], ob[:, hs]
                    )
                return
            nc.vector._custom_dve(
                sqadd,
                out=ob[:, o * ST : (o + 1) * ST],
                in0=pre[:],
                in1=t2[:],
                s0=invt_sb[:],
            )
            if m == n_m - 1 and s >= n_st - OBW // ST:
                # final pair: per-supertile DMAs so the last transfer is
                # smaller and the kernel-exit barrier waits less
                nc.scalar.dma_start(
                    out[ms, s * ST : (s + 1) * ST], ob[:, o * ST : (o + 1) * ST]
                )
            elif o == OBW // ST - 1:
                s0 = s - o
                # the last m-row rides the ScalarE ring: the SP ring still
                # has ~3 queued 1 MiB transfers at the end, and the exit
                # barrier would wait on that backlog
                oeng = nc.scalar if m == n_m - 1 else nc.sync
                oeng.dma_start(out[ms, s0 * ST : s0 * ST + OBW], ob[:])

        idx = 0
        for m in range(n_m):
            for s in range(n_st):
                if m == 0 and s <= n_st - 3:
                    emit_f2(s + 2)
                supertile(idx, m, s)
                idx += 1

    nc.compile()
    return nc


def _host_prep(rx_real, rx_imag, freq_real, freq_imag, temperature, bpc=BPC, t=T):
    """Layout marshaling only: shard/transpose/cast inputs for the cores."""
    lk = np.outer(np.arange(L), np.arange(L)).astype(np.float64)
    w = np.exp(-2j * np.pi * lk / L) / np.sqrt(L)  # ortho DFT matrix (symmetric)
    w_r = np.ascontiguousarray(w.real.astype(np.float32).astype(_BF16))
    w_i = np.ascontiguousarray(w.imag.astype(np.float32).astype(_BF16))
    w_ni = np.ascontiguousarray((-w.imag).astype(np.float32).astype(_BF16))

    fr = freq_real[:t].T.astype(np.float32)  # [L, T]
    fi = freq_imag[:t].T.astype(np.float32)
    fq1 = np.ascontiguousarray(fr.astype(_BF16))
    fq3 = np.ascontiguousarray((fi - fr).astype(_BF16))
    temp_col = np.full((128, 1), np.asarray(temperature), np.float32)

    rxt_r = np.asarray(rx_real, np.float32).T.astype(_BF16)  # [L, B]
    rxt_i = np.asarray(rx_imag, np.float32).T.astype(_BF16)

    in_maps = []
    for c in range(NCORES):
        cs = slice(c * bpc, (c + 1) * bpc)
        in_maps.append(
            {
                "rxt_r": np.ascontiguousarray(rxt_r[:, cs]),
                "rxt_i": np.ascontiguousarray(rxt_i[:, cs]),
                "fq1": fq1,
                "fq3": fq3,
                "w_r": w_r,
                "w_i": w_i,
                "w_ni": w_ni,
                "temp": temp_col,
            }
        )
    return in_maps


def kernel(rx_real, rx_imag, freq_real, freq_imag, temperature):
    from concourse.bass_utils import run_bass_kernel_spmd

    if "nc" not in _CACHE:
        _CACHE["nc"] = build_nc()
    nc = _CACHE["nc"]

    in_maps = _host_prep(rx_real, rx_imag, freq_real, freq_imag, temperature)
    res = run_bass_kernel_spmd(nc, in_maps, core_ids=list(range(NCORES)))
    _CACHE["last_result"] = res
    return np.concatenate([r["out"] for r in res.results], axis=0)


# revision 13
# speedup vs baseline: 1.0366x; 1.0082x over previous
"""Trainium2 Bass kernel: batched complex-waveform similarity.

Math: reference computes
    bank = ifft_ortho(freq)                # [T, L] complex
    score = rx @ conj(bank).T              # [B, T] complex
    sim   = (score.re^2 + score.im^2) / temperature

Since the ortho DFT is unitary,  score = fft_ortho(rx) @ conj(freq).T.
So the kernel never builds the bank: it DFTs rx via a 128x128 bf16
matmul, then runs the big complex GEMM [B,L]x[L,T] in bf16 with fp32
PSUM accumulation and a fused squared-magnitude epilogue.

With f1=fr, f2=-(fr+fi), f3=fi-fr and xs=xr+xi, the complex GEMM is
    im = xs*f1 + xr*f2;  re = xs*f1 + xi*f3
(4 matmuls per [128,1024] supertile half-pair; consecutive matmuls
share the stationary operand, which lets PE issue at ~215ns/matmul
instead of ~259). im is accumulated first so ACT's Square(im) overlaps
the re matmuls; DVE then fuses out = (re^2 + t2) * (1/temp).

PSUM: four [128,1024] tiles (reA/imA for even supertiles, reB/imB for
odd) = all 8 banks; a supertile's banks are reused 2 supertiles later,
which gives the epilogue a full cadence to drain.

Sharding: data-parallel over the rx batch dim across 8 NeuronCores;
freq planes are replicated on every core.
"""

import numpy as np
import ml_dtypes

B = 8192
T = 8192
L = 128
NCORES = 8
BPC = B // NCORES  # batch rows per core

_BF16 = ml_dtypes.bfloat16

_CACHE = {}


# --------------------------------------------------------------------------- #
# Custom DVE op: out = (Src0^2 + Src1) * C0
# (Src0 = re from PSUM, Src1 = im^2 staged by ScalarE, C0 = 1/temperature)
# --------------------------------------------------------------------------- #
def _get_sqadd_op():
    import concourse.dve_ops as dve_ops
    from concourse.dve_spec import Spec, Src0, Src1, C0, sq, lower, _has_src1
    from concourse.dve_uop import DveOpSpec

    name = "SQ_ADD_SCALE_ANT"
    for op in dve_ops.OPS:
        if op.name == name:
            return op

    spec = Spec(
        body=(sq(Src0) + Src1) * C0,
        reference=lambda in0, in1, s0, s1, imm2: (
            (in0.astype(np.float32) ** 2 + in1.astype(np.float32)) * s0
        ).astype(np.float32),
    )
    opcode = dve_ops._CUSTOM_DVE_ROW_BASE + len(dve_ops.OPS)
    assert opcode < 0x20
    shas = {}
    for ver in ("v3", "v4"):
        compiled = DveOpSpec(
            name=name, opcode=opcode, uops=lower(spec, ver=ver), rd1_en=_has_src1(spec)
        )
        shas[ver] = compiled.sha(ver)
    op = dve_ops.DveOp(name, spec, subdim=False, uops_sha=shas)
    dve_ops.OPS.append(op)
    dve_ops.CUSTOM_DVE_SPECS[name] = spec
    dve_ops._SUB_OPCODE_FOR_NAME[name] = opcode
    return op


# --------------------------------------------------------------------------- #
# Bass program (one SPMD NeuronCore)
# --------------------------------------------------------------------------- #
def build_nc(bpc=BPC, t=T, debug=False):
    from contextlib import ExitStack

    import concourse.bacc as bacc
    import concourse.bass as bass
    import concourse.mybir as mybir
    import concourse.tile as tile

    f32 = mybir.dt.float32
    bf16 = mybir.dt.bfloat16
    sqadd = _get_sqadd_op()

    ST = 1024  # supertile width: 2 PSUM banks
    OBW = 2048  # out staging tile: 2 supertiles per 1 MiB DMA
    n_st = t // ST
    n_m = bpc // 128
    assert bpc % 512 == 0 and t % ST == 0

    nc = bacc.Bacc("TRN2", target_bir_lowering=False, debug=debug, num_devices=NCORES)

    rxt_r = nc.dram_tensor("rxt_r", [L, bpc], bf16, kind="ExternalInput")
    rxt_i = nc.dram_tensor("rxt_i", [L, bpc], bf16, kind="ExternalInput")
    fq1 = nc.dram_tensor("fq1", [L, t], bf16, kind="ExternalInput")  # fr
    fq3 = nc.dram_tensor("fq3", [L, t], bf16, kind="ExternalInput")  # fi-fr
    w_r = nc.dram_tensor("w_r", [L, L], bf16, kind="ExternalInput")
    w_i = nc.dram_tensor("w_i", [L, L], bf16, kind="ExternalInput")
    w_ni = nc.dram_tensor("w_ni", [L, L], bf16, kind="ExternalInput")
    temp = nc.dram_tensor("temp", [128, 1], f32, kind="ExternalInput")
    out = nc.dram_tensor("out", [bpc, t], f32, kind="ExternalOutput")

    with tile.TileContext(nc) as tc, ExitStack() as ctx:
        consts = ctx.enter_context(tc.tile_pool(name="consts", bufs=1))
        psum = ctx.enter_context(
            tc.tile_pool(name="psum", bufs=1, space=bass.MemorySpace.PSUM)
        )
        sq_pool = ctx.enter_context(tc.tile_pool(name="sq", bufs=4))
        out_pool = ctx.enter_context(tc.tile_pool(name="ob", bufs=4))

        def ptile(k, which):  # which in ("re", "im")
            tag = f"{which}{'AB'[k % 2]}"
            return psum.tile([128, ST], f32, tag=tag, name=f"ps_{tag}")

        # ---- PE warmup ------------------------------------------------ #
        # Dependency-free matmuls run during the input-DMA window so the
        # HAM clock gate is at 8/8 when the real matmuls start (~12.5us).
        warm_w = consts.tile([128, 128], bf16)
        nc.gpsimd.memset(warm_w[:], 0)
        # dummy Pool tensor-op: loads the Q7 library now, not on the
        # critical xs = xr + xi adds after the DFT
        nc.gpsimd.tensor_add(warm_w[:, 0:1], warm_w[:, 0:1], warm_w[:, 0:1])
        warm_ps = ptile(0, "re")
        for _ in range(44):
            nc.tensor.matmul(warm_ps[:, 0:128], warm_w[:], warm_w[:], start=True, stop=True)

        # ---- load constants / inputs ---------------------------------- #
        # rx first on the SP ring; W + temp first on the ScalarE ring (in
        # parallel), then the freq planes interleave across both rings so
        # group g lands well before the m=0 walk reaches it.
        rxr_sb = consts.tile([L, bpc], bf16)
        nc.sync.dma_start(rxr_sb[:], rxt_r[:, :])
        rxi_sb = consts.tile([L, bpc], bf16)
        nc.sync.dma_start(rxi_sb[:], rxt_i[:, :])
        wr_sb = consts.tile([L, L], bf16)
        nc.sync.dma_start(wr_sb[:], w_r[:, :])
        wni_sb = consts.tile([L, L], bf16)
        nc.sync.dma_start(wni_sb[:], w_ni[:, :])
        wi_sb = consts.tile([L, L], bf16)
        nc.sync.dma_start(wi_sb[:], w_i[:, :])
        temp_sb = consts.tile([128, 1], f32)
        nc.scalar.dma_start(temp_sb[:], temp[:, :])

        f1_sb, f2_sb, f3_sb = [], [], []
        for g in range(n_st):
            gs = slice(g * ST, (g + 1) * ST)
            for plane, dram, lst in (("f1", fq1, f1_sb), ("f3", fq3, f3_sb)):
                ft = consts.tile([L, ST], bf16, tag=f"{plane}g{g}", name=f"{plane}g{g}")
                nc.sync.dma_start(ft[:], dram[:, gs])
                lst.append(ft)
            # f2 = -(fr+fi) = -(2*f1 + f3), computed on DVE once f1/f3 land
            f2_sb.append(consts.tile([L, ST], bf16, tag=f"f2g{g}", name=f"f2g{g}"))

        def emit_f2(g):
            nc.vector.scalar_tensor_tensor(
                f2_sb[g][:], f1_sb[g][:], -2.0, f3_sb[g][:],
                mybir.AluOpType.mult, mybir.AluOpType.subtract,
            )

        invt_sb = consts.tile([128, 1], f32)
        nc.vector.reciprocal(invt_sb[:], temp_sb[:])

        # ---- DFT of rx (bf16): rxfT = W @ rxT ------------------------- #
        # W symmetric, so PE's lhsT is W itself.
        # rxfT_r = Wr@rxT_r - Wi@rxT_i ; rxfT_i = Wr@rxT_i + Wi@rxT_r
        # Evictions: ACT -> xr, DVE -> xi, Pool -> xs = xr + xi.
        xr_sb = consts.tile([L, bpc], bf16)
        xi_sb = consts.tile([L, bpc], bf16)
        xs_sb = consts.tile([L, bpc], bf16)
        for ci, c0 in enumerate(range(0, bpc, 512)):
            cs = slice(c0, c0 + 512)
            pr = ptile(1, "re")
            nc.tensor.matmul(pr[:, 0:512], wr_sb[:], rxr_sb[:, cs], start=True, stop=False)
            nc.tensor.matmul(pr[:, 0:512], wni_sb[:], rxi_sb[:, cs], start=False, stop=True)
            pi = ptile(1, "im")
            nc.tensor.matmul(pi[:, 0:512], wr_sb[:], rxi_sb[:, cs], start=True, stop=False)
            nc.tensor.matmul(pi[:, 0:512], wi_sb[:], rxr_sb[:, cs], start=False, stop=True)
            # chunk 0's evictions split 256-wide so xs[:, :128] (all the
            # m=0 row needs) is ready as early as possible
            splits = (0, 256, 512) if ci == 0 else (0, 512)
            for e0, e1 in zip(splits, splits[1:]):
                es = slice(c0 + e0, c0 + e1)
                ps = slice(e0, e1)
                nc.scalar.copy(xr_sb[:, es], pr[:, ps])
                if ci == 0:
                    nc.vector.tensor_copy(xi_sb[:, es], pi[:, ps])
                else:
                    nc.scalar.copy(xi_sb[:, es], pi[:, ps])
                nc.gpsimd.tensor_add(xs_sb[:, es], xr_sb[:, es], xi_sb[:, es])
        emit_f2(0)
        emit_f2(1)

        # ---- main complex GEMM + fused |.|^2 epilogue ----------------- #
        # Pure 4-matmul complex GEMM per [128,1024] supertile, im banks
        # first so ACT's square overlaps the re matmuls, then DVE's fused
        # (re^2 + t2) * (1/temp) drains both banks.  Each supertile's
        # banks recycle 2 supertiles later (reA/imA vs reB/imB), which
        # just covers the mm->square->sqadd chain.
        state = {"ob": None}

        def supertile(idx, m, s):
            ms = slice(m * 128, (m + 1) * 128)
            pim = ptile(idx, "im")
            pre = ptile(idx, "re")
            for h in (0, 1):
                hs = slice(h * 512, (h + 1) * 512)
                nc.tensor.matmul(
                    pim[:, hs], xs_sb[:, ms], f1_sb[s][:, hs],
                    start=True, stop=True, skip_group_check=True,
                )
            for h in (0, 1):
                hs = slice(h * 512, (h + 1) * 512)
                nc.tensor.matmul(
                    pim[:, hs], xr_sb[:, ms], f2_sb[s][:, hs],
                    start=False, stop=True, skip_group_check=True,
                )
            t2 = sq_pool.tile([128, ST], f32, name="t2")
            nc.scalar.square(t2[:], pim[:])
            for h in (0, 1):
                hs = slice(h * 512, (h + 1) * 512)
                nc.tensor.matmul(
                    pre[:, hs], xs_sb[:, ms], f1_sb[s][:, hs],
                    start=True, stop=True, skip_group_check=True,
                )
            for h in (0, 1):
                hs = slice(h * 512, (h + 1) * 512)
                nc.tensor.matmul(
                    pre[:, hs], xi_sb[:, ms], f3_sb[s][:, hs],
                    start=False, stop=True, skip_group_check=True,
                )
            o = s % (OBW // ST)
            if o == 0:
                state["ob"] = out_pool.tile([128, OBW], f32, name="ob")
            ob = state["ob"]
            nc.vector._custom_dve(
                sqadd,
                out=ob[:, o * ST : (o + 1) * ST],
                in0=pre[:],
                in1=t2[:],
                s0=invt_sb[:],
            )
            if m == n_m - 1 and s >= n_st - OBW // ST:
                # final pair: per-supertile DMAs so the last transfer is
                # smaller and the kernel-exit barrier waits less
                nc.scalar.dma_start(
                    out[ms, s * ST : (s + 1) * ST], ob[:, o * ST : (o + 1) * ST]
                )
            elif o == OBW // ST - 1:
                s0 = s - o
                # the last m-row rides the ScalarE ring: the SP ring still
                # has ~3 queued 1 MiB transfers at the end, and the exit
                # barrier would wait on that backlog
                oeng = nc.scalar if m == n_m - 1 else nc.sync
                oeng.dma_start(out[ms, s0 * ST : s0 * ST + OBW], ob[:])

        idx = 0
        for m in range(n_m):
            for s in range(n_st):
                if m == 0 and s <= n_st - 3:
                    emit_f2(s + 2)
                supertile(idx, m, s)
                idx += 1

    nc.compile()
    return nc


def _host_prep(rx_real, rx_imag, freq_real, freq_imag, temperature, bpc=BPC, t=T):
    """Layout marshaling only: shard/transpose/cast inputs for the cores."""
    lk = np.outer(np.arange(L), np.arange(L)).astype(np.float64)
    w = np.exp(-2j * np.pi * lk / L) / np.sqrt(L)  # ortho DFT matrix (symmetric)
    w_r = np.ascontiguousarray(w.real.astype(np.float32).astype(_BF16))
    w_i = np.ascontiguousarray(w.imag.astype(np.float32).astype(_BF16))
    w_ni = np.ascontiguousarray((-w.imag).astype(np.float32).astype(_BF16))

    fr = freq_real[:t].T.astype(np.float32)  # [L, T]
    fi = freq_imag[:t].T.astype(np.float32)
    fq1 = np.ascontiguousarray(fr.astype(_BF16))
    fq3 = np.ascontiguousarray((fi - fr).astype(_BF16))
    temp_col = np.full((128, 1), np.asarray(temperature), np.float32)

    rxt_r = np.asarray(rx_real, np.float32).T.astype(_BF16)  # [L, B]
    rxt_i = np.asarray(rx_imag, np.float32).T.astype(_BF16)

    in_maps = []
    for c in range(NCORES):
        cs = slice(c * bpc, (c + 1) * bpc)
        in_maps.append(
            {
                "rxt_r": np.ascontiguousarray(rxt_r[:, cs]),
                "rxt_i": np.ascontiguousarray(rxt_i[:, cs]),
                "fq1": fq1,
                "fq3": fq3,
                "w_r": w_r,
                "w_i": w_i,
                "w_ni": w_ni,
                "temp": temp_col,
            }
        )
    return in_maps


def kernel(rx_real, rx_imag, freq_real, freq_imag, temperature):
    from concourse.bass_utils import run_bass_kernel_spmd

    if "nc" not in _CACHE:
        _CACHE["nc"] = build_nc()
    nc = _CACHE["nc"]

    in_maps = _host_prep(rx_real, rx_imag, freq_real, freq_imag, temperature)
    res = run_bass_kernel_spmd(nc, in_maps, core_ids=list(range(NCORES)))
    _CACHE["last_result"] = res
    return np.concatenate([r["out"] for r in res.results], axis=0)


# revision 14
# speedup vs baseline: 1.0469x; 1.0099x over previous
"""Trainium2 Bass kernel: batched complex-waveform similarity.

Math: reference computes
    bank = ifft_ortho(freq)                # [T, L] complex
    score = rx @ conj(bank).T              # [B, T] complex
    sim   = (score.re^2 + score.im^2) / temperature

Since the ortho DFT is unitary,  score = fft_ortho(rx) @ conj(freq).T.
So the kernel never builds the bank: it DFTs rx via a 128x128 bf16
matmul, then runs the big complex GEMM [B,L]x[L,T] in bf16 with fp32
PSUM accumulation and a fused squared-magnitude epilogue.

With f1=fr, f2=-(fr+fi), f3=fi-fr and xs=xr+xi, the complex GEMM is
    im = xs*f1 + xr*f2;  re = xs*f1 + xi*f3
(4 matmuls per [128,1024] supertile half-pair; consecutive matmuls
share the stationary operand, which lets PE issue at ~215ns/matmul
instead of ~259). im is accumulated first so ACT's Square(im) overlaps
the re matmuls; DVE then fuses out = (re^2 + t2) * (1/temp).

PSUM: four [128,1024] tiles (reA/imA for even supertiles, reB/imB for
odd) = all 8 banks; a supertile's banks are reused 2 supertiles later,
which gives the epilogue a full cadence to drain.

Sharding: data-parallel over the rx batch dim across 8 NeuronCores;
freq planes are replicated on every core.
"""

import numpy as np
import ml_dtypes

B = 8192
T = 8192
L = 128
NCORES = 8
BPC = B // NCORES  # batch rows per core

_BF16 = ml_dtypes.bfloat16

_CACHE = {}


# --------------------------------------------------------------------------- #
# Custom DVE op: out = (Src0^2 + Src1) * C0
# (Src0 = re from PSUM, Src1 = im^2 staged by ScalarE, C0 = 1/temperature)
# --------------------------------------------------------------------------- #
def _get_sqadd_op():
    import concourse.dve_ops as dve_ops
    from concourse.dve_spec import Spec, Src0, Src1, C0, sq, lower, _has_src1
    from concourse.dve_uop import DveOpSpec

    name = "SQ_ADD_SCALE_ANT"
    for op in dve_ops.OPS:
        if op.name == name:
            return op

    spec = Spec(
        body=(sq(Src0) + Src1) * C0,
        reference=lambda in0, in1, s0, s1, imm2: (
            (in0.astype(np.float32) ** 2 + in1.astype(np.float32)) * s0
        ).astype(np.float32),
    )
    opcode = dve_ops._CUSTOM_DVE_ROW_BASE + len(dve_ops.OPS)
    assert opcode < 0x20
    shas = {}
    for ver in ("v3", "v4"):
        compiled = DveOpSpec(
            name=name, opcode=opcode, uops=lower(spec, ver=ver), rd1_en=_has_src1(spec)
        )
        shas[ver] = compiled.sha(ver)
    op = dve_ops.DveOp(name, spec, subdim=False, uops_sha=shas)
    dve_ops.OPS.append(op)
    dve_ops.CUSTOM_DVE_SPECS[name] = spec
    dve_ops._SUB_OPCODE_FOR_NAME[name] = opcode
    return op


# --------------------------------------------------------------------------- #
# Bass program (one SPMD NeuronCore)
# --------------------------------------------------------------------------- #
def build_nc(bpc=BPC, t=T, debug=False):
    from contextlib import ExitStack

    import concourse.bacc as bacc
    import concourse.bass as bass
    import concourse.mybir as mybir
    import concourse.tile as tile

    f32 = mybir.dt.float32
    bf16 = mybir.dt.bfloat16
    sqadd = _get_sqadd_op()

    ST = 1024  # supertile width: 2 PSUM banks
    OBW = 2048  # out staging tile: 2 supertiles per 1 MiB DMA
    n_st = t // ST
    n_m = bpc // 128
    assert bpc % 512 == 0 and t % ST == 0

    nc = bacc.Bacc("TRN2", target_bir_lowering=False, debug=debug, num_devices=NCORES)

    rxt_r = nc.dram_tensor("rxt_r", [L, bpc], bf16, kind="ExternalInput")
    rxt_i = nc.dram_tensor("rxt_i", [L, bpc], bf16, kind="ExternalInput")
    fq1 = nc.dram_tensor("fq1", [L, t], bf16, kind="ExternalInput")  # fr
    fq3 = nc.dram_tensor("fq3", [L, t], bf16, kind="ExternalInput")  # fi-fr
    fq2h = nc.dram_tensor("fq2h", [L, t // 2], bf16, kind="ExternalInput")  # -(fr+fi), back half
    w_r = nc.dram_tensor("w_r", [L, L], bf16, kind="ExternalInput")
    w_i = nc.dram_tensor("w_i", [L, L], bf16, kind="ExternalInput")
    w_ni = nc.dram_tensor("w_ni", [L, L], bf16, kind="ExternalInput")
    temp = nc.dram_tensor("temp", [128, 1], f32, kind="ExternalInput")
    out = nc.dram_tensor("out", [bpc, t], f32, kind="ExternalOutput")

    with tile.TileContext(nc) as tc, ExitStack() as ctx:
        consts = ctx.enter_context(tc.tile_pool(name="consts", bufs=1))
        psum = ctx.enter_context(
            tc.tile_pool(name="psum", bufs=1, space=bass.MemorySpace.PSUM)
        )
        sq_pool = ctx.enter_context(tc.tile_pool(name="sq", bufs=4))
        out_pool = ctx.enter_context(tc.tile_pool(name="ob", bufs=4))

        def ptile(k, which):  # which in ("re", "im")
            tag = f"{which}{'AB'[k % 2]}"
            return psum.tile([128, ST], f32, tag=tag, name=f"ps_{tag}")

        # ---- PE warmup ------------------------------------------------ #
        # Dependency-free matmuls run during the input-DMA window so the
        # HAM clock gate is at 8/8 when the real matmuls start (~12.5us).
        warm_w = consts.tile([128, 128], bf16)
        nc.gpsimd.memset(warm_w[:], 0)
        # dummy Pool tensor-op: loads the Q7 library now, not on the
        # critical xs = xr + xi adds after the DFT
        nc.gpsimd.tensor_add(warm_w[:, 0:1], warm_w[:, 0:1], warm_w[:, 0:1])
        warm_ps = ptile(0, "re")
        for _ in range(44):
            nc.tensor.matmul(warm_ps[:, 0:128], warm_w[:], warm_w[:], start=True, stop=True)

        # ---- load constants / inputs ---------------------------------- #
        # rx first on the SP ring; W + temp first on the ScalarE ring (in
        # parallel), then the freq planes interleave across both rings so
        # group g lands well before the m=0 walk reaches it.
        rxr_sb = consts.tile([L, bpc], bf16)
        nc.sync.dma_start(rxr_sb[:], rxt_r[:, :])
        rxi_sb = consts.tile([L, bpc], bf16)
        nc.sync.dma_start(rxi_sb[:], rxt_i[:, :])
        wr_sb = consts.tile([L, L], bf16)
        nc.sync.dma_start(wr_sb[:], w_r[:, :])
        wni_sb = consts.tile([L, L], bf16)
        nc.sync.dma_start(wni_sb[:], w_ni[:, :])
        wi_sb = consts.tile([L, L], bf16)
        nc.sync.dma_start(wi_sb[:], w_i[:, :])
        temp_sb = consts.tile([128, 1], f32)
        nc.scalar.dma_start(temp_sb[:], temp[:, :])

        f1_sb, f2_sb, f3_sb = [], [], []
        for g in range(n_st):
            gs = slice(g * ST, (g + 1) * ST)
            for plane, dram, lst in (("f1", fq1, f1_sb), ("f3", fq3, f3_sb)):
                ft = consts.tile([L, ST], bf16, tag=f"{plane}g{g}", name=f"{plane}g{g}")
                nc.sync.dma_start(ft[:], dram[:, gs])
                lst.append(ft)
            # f2 = -(fr+fi) = -(2*f1 + f3): groups 0..3 computed on DVE
            # (their host copies would not load in time), groups 4..7
            # DMA'd from the host (they land before the m=0 walk arrives)
            f2t = consts.tile([L, ST], bf16, tag=f"f2g{g}", name=f"f2g{g}")
            if g >= n_st // 2:
                h0 = (g - n_st // 2) * ST
                nc.sync.dma_start(f2t[:], fq2h[:, h0 : h0 + ST])
            f2_sb.append(f2t)

        def emit_f2(g):
            nc.vector.scalar_tensor_tensor(
                f2_sb[g][:], f1_sb[g][:], -2.0, f3_sb[g][:],
                mybir.AluOpType.mult, mybir.AluOpType.subtract,
            )

        invt_sb = consts.tile([128, 1], f32)
        nc.vector.reciprocal(invt_sb[:], temp_sb[:])

        # ---- DFT of rx (bf16): rxfT = W @ rxT ------------------------- #
        # W symmetric, so PE's lhsT is W itself.
        # rxfT_r = Wr@rxT_r - Wi@rxT_i ; rxfT_i = Wr@rxT_i + Wi@rxT_r
        # Evictions: ACT -> xr, DVE -> xi, Pool -> xs = xr + xi.
        xr_sb = consts.tile([L, bpc], bf16)
        xi_sb = consts.tile([L, bpc], bf16)
        xs_sb = consts.tile([L, bpc], bf16)
        for ci, c0 in enumerate(range(0, bpc, 512)):
            cs = slice(c0, c0 + 512)
            pr = ptile(1, "re")
            nc.tensor.matmul(pr[:, 0:512], wr_sb[:], rxr_sb[:, cs], start=True, stop=False)
            nc.tensor.matmul(pr[:, 0:512], wni_sb[:], rxi_sb[:, cs], start=False, stop=True)
            pi = ptile(1, "im")
            nc.tensor.matmul(pi[:, 0:512], wr_sb[:], rxi_sb[:, cs], start=True, stop=False)
            nc.tensor.matmul(pi[:, 0:512], wi_sb[:], rxr_sb[:, cs], start=False, stop=True)
            # chunk 0's evictions split 256-wide so xs[:, :128] (all the
            # m=0 row needs) is ready as early as possible
            splits = (0, 256, 512) if ci == 0 else (0, 512)
            for e0, e1 in zip(splits, splits[1:]):
                es = slice(c0 + e0, c0 + e1)
                ps = slice(e0, e1)
                nc.scalar.copy(xr_sb[:, es], pr[:, ps])
                if ci == 0:
                    nc.vector.tensor_copy(xi_sb[:, es], pi[:, ps])
                else:
                    nc.scalar.copy(xi_sb[:, es], pi[:, ps])
                nc.gpsimd.tensor_add(xs_sb[:, es], xr_sb[:, es], xi_sb[:, es])
        emit_f2(0)
        emit_f2(1)

        # ---- main complex GEMM + fused |.|^2 epilogue ----------------- #
        # Pure 4-matmul complex GEMM per [128,1024] supertile, im banks
        # first so ACT's square overlaps the re matmuls, then DVE's fused
        # (re^2 + t2) * (1/temp) drains both banks.  Each supertile's
        # banks recycle 2 supertiles later (reA/imA vs reB/imB), which
        # just covers the mm->square->sqadd chain.
        state = {"ob": None}

        def supertile(idx, m, s):
            ms = slice(m * 128, (m + 1) * 128)
            pim = ptile(idx, "im")
            pre = ptile(idx, "re")
            for h in (0, 1):
                hs = slice(h * 512, (h + 1) * 512)
                nc.tensor.matmul(
                    pim[:, hs], xs_sb[:, ms], f1_sb[s][:, hs],
                    start=True, stop=True, skip_group_check=True,
                )
            for h in (0, 1):
                hs = slice(h * 512, (h + 1) * 512)
                nc.tensor.matmul(
                    pim[:, hs], xr_sb[:, ms], f2_sb[s][:, hs],
                    start=False, stop=True, skip_group_check=True,
                )
            t2 = sq_pool.tile([128, ST], f32, name="t2")
            nc.scalar.square(t2[:], pim[:])
            for h in (0, 1):
                hs = slice(h * 512, (h + 1) * 512)
                nc.tensor.matmul(
                    pre[:, hs], xs_sb[:, ms], f1_sb[s][:, hs],
                    start=True, stop=True, skip_group_check=True,
                )
            for h in (0, 1):
                hs = slice(h * 512, (h + 1) * 512)
                nc.tensor.matmul(
                    pre[:, hs], xi_sb[:, ms], f3_sb[s][:, hs],
                    start=False, stop=True, skip_group_check=True,
                )
            o = s % (OBW // ST)
            if o == 0:
                state["ob"] = out_pool.tile([128, OBW], f32, name="ob")
            ob = state["ob"]
            nc.vector._custom_dve(
                sqadd,
                out=ob[:, o * ST : (o + 1) * ST],
                in0=pre[:],
                in1=t2[:],
                s0=invt_sb[:],
            )
            if m == n_m - 1 and s >= n_st - OBW // ST:
                # final pair: per-supertile DMAs so the last transfer is
                # smaller and the kernel-exit barrier waits less
                nc.scalar.dma_start(
                    out[ms, s * ST : (s + 1) * ST], ob[:, o * ST : (o + 1) * ST]
                )
            elif o == OBW // ST - 1:
                s0 = s - o
                # the last m-row rides the ScalarE ring: the SP ring still
                # has ~3 queued 1 MiB transfers at the end, and the exit
                # barrier would wait on that backlog
                oeng = nc.scalar if m == n_m - 1 else nc.sync
                oeng.dma_start(out[ms, s0 * ST : s0 * ST + OBW], ob[:])

        idx = 0
        for m in range(n_m):
            for s in range(n_st):
                if m == 0 and 2 + s < n_st // 2:
                    emit_f2(s + 2)
                supertile(idx, m, s)
                idx += 1

    nc.compile()
    return nc


def _host_prep(rx_real, rx_imag, freq_real, freq_imag, temperature, bpc=BPC, t=T):
    """Layout marshaling only: shard/transpose/cast inputs for the cores."""
    lk = np.outer(np.arange(L), np.arange(L)).astype(np.float64)
    w = np.exp(-2j * np.pi * lk / L) / np.sqrt(L)  # ortho DFT matrix (symmetric)
    w_r = np.ascontiguousarray(w.real.astype(np.float32).astype(_BF16))
    w_i = np.ascontiguousarray(w.imag.astype(np.float32).astype(_BF16))
    w_ni = np.ascontiguousarray((-w.imag).astype(np.float32).astype(_BF16))

    fr = freq_real[:t].T.astype(np.float32)  # [L, T]
    fi = freq_imag[:t].T.astype(np.float32)
    fq1 = np.ascontiguousarray(fr.astype(_BF16))
    fq3 = np.ascontiguousarray((fi - fr).astype(_BF16))
    fq2h = np.ascontiguousarray((-(fr + fi))[:, t // 2 :].astype(_BF16))
    temp_col = np.full((128, 1), np.asarray(temperature), np.float32)

    rxt_r = np.asarray(rx_real, np.float32).T.astype(_BF16)  # [L, B]
    rxt_i = np.asarray(rx_imag, np.float32).T.astype(_BF16)

    in_maps = []
    for c in range(NCORES):
        cs = slice(c * bpc, (c + 1) * bpc)
        in_maps.append(
            {
                "rxt_r": np.ascontiguousarray(rxt_r[:, cs]),
                "rxt_i": np.ascontiguousarray(rxt_i[:, cs]),
                "fq1": fq1,
                "fq3": fq3,
                "fq2h": fq2h,
                "w_r": w_r,
                "w_i": w_i,
                "w_ni": w_ni,
                "temp": temp_col,
            }
        )
    return in_maps


def kernel(rx_real, rx_imag, freq_real, freq_imag, temperature):
    from concourse.bass_utils import run_bass_kernel_spmd

    if "nc" not in _CACHE:
        _CACHE["nc"] = build_nc()
    nc = _CACHE["nc"]

    in_maps = _host_prep(rx_real, rx_imag, freq_real, freq_imag, temperature)
    res = run_bass_kernel_spmd(nc, in_maps, core_ids=list(range(NCORES)))
    _CACHE["last_result"] = res
    return np.concatenate([r["out"] for r in res.results], axis=0)
